# Initial kernel scaffold
#
"""Trainium2 Bass kernel for nn_PointCloudDiffusionModel (training-loss forward).

Reference computation: q_sample + 6-layer pointwise MLP (4->64->128->128->128->64->3)
with train-mode BatchNorm (global stats over all B*N points) + ReLU after the
first 5 layers; output = scalar MSE(pred, noise).

Distribution: pure data parallel over the B*N = 524288 points; each of the 8
cores owns 8 batches (65536 points).  Per-channel BN statistics are combined
with a tiny (1KB) AllReduce per layer.

Device algorithm (per core), all activations stored as bf16 [channels, points]:
  - Layer 1 is folded on the host: per-batch lhsT [8,64] with channel order
    [n0,n1,n2,ones,x0,x1,x2,0] absorbs q_sample scales sqrt(ac_t), sqrt(1-ac_t),
    the timestep embedding channel and b1 (via the ones channel).
  - For each BN layer i: pass A computes pre-activations a~ = W' h tile-by-tile
    (PSUM) and per-tile mean/M2 via DVE bn_stats; bn_aggr + 1KB AllReduce give
    global mean/var.  Since gamma>0:  h = relu(s*(a-m)+beta) = s*relu(a + u)
    with s = gamma/std, u = beta/s - m,  so the per-channel scale s is folded
    into the NEXT layer's weights (tiny on-device op) and pass B re-runs the
    matmul and evacuates PSUM->SBUF with a single fused Relu(a+u) activation op.
    (BN absorbs the linear bias, so b2..b5 drop out exactly.)
  - Layer 6 computes pred - noise directly via a 2-matmul PSUM accumulation
    ([W6'|(-I;b6)] against [h5] and [noise;ones]) and only its bn_stats are
    shipped out; the host combines the 8 cores' per-chunk stats into the MSE.
"""
import numpy as np
import ml_dtypes

import concourse.bass as bass
import concourse.mybir as mybir
from concourse import tile
from concourse.bass_utils import run_bass_kernel_spmd

BF16 = ml_dtypes.bfloat16
F32 = mybir.dt.float32
BF = mybir.dt.bfloat16
AOp = mybir.AluOpType
AF = mybir.ActivationFunctionType

NCORES = 8
NB = 8            # batches per core
TS = 512          # points per tile (= one PSUM bank of fp32)
BN_EPS = 1e-5
T_STEPS = 1000

# wm / wf column layout (lhsT = [C_in, C_out] slices)
COLS = {2: (0, 128, 64), 3: (128, 256, 128), 4: (256, 384, 128),
        5: (384, 448, 128), 6: (448, 451, 64)}  # start, stop, n_rows
C_OUT = {1: 64, 2: 128, 3: 128, 4: 128, 5: 64, 6: 3}


def _schedule():
    s = 0.008
    x = np.linspace(0.0, T_STEPS, T_STEPS + 1)
    ac = np.cos((x / T_STEPS + s) / (1.0 + s) * np.pi * 0.5) ** 2
    ac = ac / ac[0]
    betas = np.clip(1.0 - ac[1:] / ac[:-1], 1e-4, 2e-2)
    acp = np.cumprod(1.0 - betas)
    return np.sqrt(acp).astype(np.float32), np.sqrt(1.0 - acp).astype(np.float32)


def build_program(nc: bass.Bass, ptb: int):
    """Build the SPMD per-core program.  ptb = points per batch (N)."""
    ppc = NB * ptb                 # points per core
    nt_b = ptb // TS               # tiles per batch
    ntiles = ppc // TS
    RG = [list(range(NCORES))]

    xn_d = nc.dram_tensor("xn", [128, 2 * ptb], BF, kind="ExternalInput")
    w1eff_d = nc.dram_tensor("w1eff", [128, 128], BF, kind="ExternalInput")
    wm_d = nc.dram_tensor("wm", [128, 452], F32, kind="ExternalInput")
    w6b_d = nc.dram_tensor("w6b", [128, 4], BF, kind="ExternalInput")
    bnp_d = nc.dram_tensor("bnp", [128, 10], F32, kind="ExternalInput")
    mstats_d = nc.dram_tensor("mstats", [3, 6 * ntiles], F32, kind="ExternalOutput")

    with tile.TileContext(nc) as tc:
        with (
            tc.tile_pool(name="const", bufs=1) as cpool,
            tc.tile_pool(name="work", bufs=2) as wpool,
            tc.tile_pool(name="psum", bufs=8, space="PSUM") as pspool,
            tc.tile_pool(name="dram", bufs=2, space="DRAM") as dpool,
        ):
            H = cpool.tile([128, ppc], BF)
            xn = cpool.tile([128, 2 * ptb], BF)
            w1eff = cpool.tile([128, 128], BF)
            wm = cpool.tile([128, 452], F32)
            wf = cpool.tile([128, 452], BF)
            w6b = cpool.tile([128, 4], BF)
            bnp = cpool.tile([128, 10], F32)
            mstats = cpool.tile([3, 6 * ntiles], F32)

            nc.sync.dma_start(xn[:, :], xn_d[:, :])
            nc.sync.dma_start(w1eff[:, :], w1eff_d[:, :])
            nc.sync.dma_start(wm[:, :], wm_d[:, :])
            nc.sync.dma_start(w6b[:, :], w6b_d[:, :])
            nc.sync.dma_start(bnp[:, :], bnp_d[:, :])

            def l1_ops(ti):
                k, tt = divmod(ti, nt_b)
                base, half = 32 * (k % 4), k // 4
                lhsT = w1eff[base:base + 8, half * 64:half * 64 + 64]
                rhs = xn[base:base + 8, half * ptb + tt * TS: half * ptb + (tt + 1) * TS]
                return lhsT, rhs

            def mk_ops(i):  # layers 2..6 main matmul operands
                lo, hi, rows = COLS[i]
                def ops(ti):
                    return wf[0:rows, lo:hi], H[0:rows, ti * TS:(ti + 1) * TS]
                return ops

            def pass_a(opfn, C, stats):
                for ti in range(ntiles):
                    lhsT, rhs = opfn(ti)
                    psa = pspool.tile([128, TS], F32, tag="ps", name=f"psa")
                    nc.tensor.matmul(psa[0:C, :], lhsT, rhs)
                    nc.vector.bn_stats(stats[0:C, 6 * ti:6 * ti + 6], psa[0:C, :])

            def pass_b(opfn, C, uv):
                for ti in range(ntiles):
                    lhsT, rhs = opfn(ti)
                    psb = pspool.tile([128, TS], F32, tag="ps", name=f"psb")
                    nc.tensor.matmul(psb[0:C, :], lhsT, rhs)
                    nc.scalar.activation(
                        H[0:C, ti * TS:(ti + 1) * TS], psb[0:C, :],
                        AF.Relu, bias=uv[0:C, :], scale=1.0)

            def stats_to_su(i, C, stats):
                """bn_aggr -> AllReduce(1KB) -> per-channel scale s and bias u."""
                agg = wpool.tile([128, 2], F32, tag="agg", name="agg")
                nc.vector.bn_aggr(agg[0:C, :], stats[0:C, 0:6 * ntiles])
                m2 = wpool.tile([128, 1], F32, tag="m2", name="m2")
                nc.vector.tensor_tensor(m2[0:C, :], agg[0:C, 0:1], agg[0:C, 0:1], op=AOp.mult)
                pk = wpool.tile([128, 2], F32, tag="pk", name="pk")
                # payload: [mean/8, (var+mean^2)/8]; AR-add over 8 equal-count cores
                nc.vector.tensor_scalar(pk[0:C, 0:1], agg[0:C, 0:1], 1.0 / NCORES, None, op0=AOp.mult)
                nc.vector.tensor_scalar(pk[0:C, 1:2], agg[0:C, 1:2], m2[0:C, :], 1.0 / NCORES,
                                        op0=AOp.add, op1=AOp.mult)
                ari = dpool.tile([C, 2], F32, tag="ari", name="ari")
                aro = dpool.tile([C, 2], F32, tag="aro", name="aro")
                nc.gpsimd.dma_start(ari[:, :], pk[0:C, :])
                nc.gpsimd.collective_compute(
                    "AllReduce", AOp.add, replica_groups=RG,
                    ins=[ari[:, :].opt()], outs=[aro[:, :].opt()])
                gb = wpool.tile([128, 2], F32, tag="gb", name="gb")
                nc.gpsimd.dma_start(gb[0:C, :], aro[:, :])
                m2g = wpool.tile([128, 1], F32, tag="m2g", name="m2g")
                nc.vector.tensor_tensor(m2g[0:C, :], gb[0:C, 0:1], gb[0:C, 0:1], op=AOp.mult)
                varv = wpool.tile([128, 1], F32, tag="varv", name="varv")
                nc.vector.tensor_tensor(varv[0:C, :], gb[0:C, 1:2], m2g[0:C, :], op=AOp.subtract)
                stdv = wpool.tile([128, 1], F32, tag="stdv", name="stdv")
                nc.scalar.activation(stdv[0:C, :], varv[0:C, :], AF.Sqrt, bias=BN_EPS, scale=1.0)
                rcpv = wpool.tile([128, 1], F32, tag="rcpv", name="rcpv")
                nc.vector.reciprocal(rcpv[0:C, :], stdv[0:C, :])
                sv = wpool.tile([128, 1], F32, tag="sv", name="sv")
                nc.vector.tensor_tensor(sv[0:C, :], rcpv[0:C, :], bnp[0:C, 2 * i - 1:2 * i], op=AOp.mult)
                uv = wpool.tile([128, 1], F32, tag="uv", name="uv")
                # u = (beta/gamma)*std - mean
                nc.vector.tensor_scalar(uv[0:C, :], stdv[0:C, :], bnp[0:C, 2 * i - 2:2 * i - 1],
                                        gb[0:C, 0:1], op0=AOp.mult, op1=AOp.subtract)
                return sv, uv

            def fold(next_i, sv):
                lo, hi, rows = COLS[next_i]
                nc.vector.tensor_scalar_mul(wf[0:rows, lo:hi], wm[0:rows, lo:hi], sv[0:rows, :])

            # ---- layers 1..5 ----
            opfns = {1: l1_ops, 2: mk_ops(2), 3: mk_ops(3), 4: mk_ops(4), 5: mk_ops(5)}
            for i in range(1, 6):
                C = C_OUT[i]
                stats = wpool.tile([128, 6 * ntiles], F32, tag="stats", name=f"stats{i}")
                pass_a(opfns[i], C, stats)
                sv, uv = stats_to_su(i, C, stats)
                fold(i + 1, sv)
                pass_b(opfns[i], C, uv)

            # ---- layer 6: pred - noise, stats only ----
            for ti in range(ntiles):
                k, tt = divmod(ti, nt_b)
                base, half = 32 * (k % 4), k // 4
                ps6 = pspool.tile([128, TS], F32, tag="ps", name="ps6")
                nc.tensor.matmul(ps6[0:3, :], wf[0:64, 448:451],
                                 H[0:64, ti * TS:(ti + 1) * TS], start=True, stop=False)
                nc.tensor.matmul(ps6[0:3, :], w6b[base:base + 4, 0:3],
                                 xn[base:base + 4, half * ptb + tt * TS: half * ptb + (tt + 1) * TS],
                                 start=False, stop=True)
                nc.vector.bn_stats(mstats[0:3, 6 * ti:6 * ti + 6], ps6[0:3, :])

            nc.sync.dma_start(mstats_d[:, :], mstats[:, :])

    return nc


# ---------------- host side ----------------

def host_pack(x, t, noise, params, ptb):
    """Build the 8 per-core input maps."""
    B, N, _ = x.shape
    assert N == ptb and B == NCORES * NB
    p = {k: np.asarray(v, np.float32) for k, v in params.items()}
    sqa, sq1 = _schedule()
    s0 = sqa[np.asarray(t)]
    s1 = sq1[np.asarray(t)]
    tf = np.asarray(t).astype(np.float32)

    w1 = p['w1']  # (64, 4)

    # wm: master lhsT weights, fp32 (identical on all cores)
    wm = np.zeros((128, 452), np.float32)
    wm[0:64, 0:128] = p['w2'].T
    wm[0:128, 128:256] = p['w3'].T
    wm[0:128, 256:384] = p['w4'].T
    wm[0:128, 384:448] = p['w5'].T
    wm[0:64, 448:451] = p['w6'].T

    # w6b: [-I3; b6] at each 32-aligned base
    w6b = np.zeros((128, 4), np.float32)
    for j in range(4):
        w6b[32 * j:32 * j + 3, 0:3] = -np.eye(3, dtype=np.float32)
        w6b[32 * j + 3, 0:3] = p['b6']
    w6b = w6b.astype(BF16)

    # bnp: per layer cols [beta/gamma, gamma]
    bnp = np.zeros((128, 10), np.float32)
    for i, (g, be) in enumerate([(p['g1'], p['be1']), (p['g2'], p['be2']),
                                 (p['g3'], p['be3']), (p['g4'], p['be4']),
                                 (p['g5'], p['be5'])]):
        C = len(g)
        bnp[0:C, 2 * i] = be / g
        bnp[0:C, 2 * i + 1] = g

    wm_l = wm
    in_maps = []
    for c in range(NCORES):
        xn = np.zeros((128, 2 * ptb), np.float32)
        w1eff = np.zeros((128, 128), np.float32)
        for k in range(NB):
            b = c * NB + k
            base, half = 32 * (k % 4), k // 4
            cols = slice(half * ptb, (half + 1) * ptb)
            xn[base + 0:base + 3, cols] = noise[b].T
            xn[base + 3, cols] = 1.0
            xn[base + 4:base + 7, cols] = x[b].T
            # lhsT [8, 64]
            blk = np.zeros((8, 64), np.float32)
            blk[0:3] = s1[b] * w1[:, 0:3].T
            blk[3] = tf[b] * w1[:, 3] + p['b1']
            blk[4:7] = s0[b] * w1[:, 0:3].T
            w1eff[base:base + 8, half * 64:(half + 1) * 64] = blk
        in_maps.append({
            "xn": xn.astype(BF16),
            "w1eff": w1eff.astype(BF16),
            "wm": wm_l,
            "w6b": w6b,
            "bnp": bnp,
        })
    return in_maps


def combine_mstats(results):
    """results: list of per-core dicts with 'mstats' [3, 6*ntiles] -> scalar MSE."""
    vals = []
    for r in results:
        ms = np.asarray(r["mstats"], np.float64)         # [3, 6*nt]
        tri = ms.reshape(3, -1, 3)                        # [3, 2*nt, 3] triples
        counts, means, ctv = tri[..., 0], tri[..., 1], tri[..., 2]
        assert np.all(counts == counts.flat[0])
        vals.append(ctv / counts + means ** 2)            # E[d^2] per equal chunk
    return np.float32(np.mean(np.concatenate([v.ravel() for v in vals])))


_PROGRAMS = {}


def get_program(ptb):
    if ptb not in _PROGRAMS:
        nc = bass.Bass(num_devices=NCORES)
        build_program(nc, ptb)
        _PROGRAMS[ptb] = nc
    return _PROGRAMS[ptb]


def kernel(x, t, noise, params, _trace=False):
    x = np.asarray(x, np.float32)
    noise = np.asarray(noise, np.float32)
    t = np.asarray(t)
    ptb = x.shape[1]
    nc = get_program(ptb)
    in_maps = host_pack(x, t, noise, params, ptb)
    res = run_bass_kernel_spmd(nc, in_maps, core_ids=list(range(NCORES)), trace=_trace)
    mse = combine_mstats(res.results)
    if _trace:
        return mse, res
    return mse


# revision 13
# speedup vs baseline: 1.9708x; 1.9708x over previous
"""Trainium2 Bass kernel for nn_PointCloudDiffusionModel (training-loss forward).

Reference computation: q_sample + 6-layer pointwise MLP (4->64->128->128->128->64->3)
with train-mode BatchNorm (global stats over all B*N points) + ReLU after the
first 5 layers; output = scalar MSE(pred, noise).

Distribution: pure data parallel over the B*N = 524288 points; each of the 8
cores owns 8 batches (65536 points).  Per-channel BN statistics are combined
with a tiny (1KB) AllReduce per layer.

Device algorithm (per core), all activations stored as bf16 [channels, points]:
  - Layer 1 is folded on the host: per-batch lhsT [8,64] with channel order
    [n0,n1,n2,ones,x0,x1,x2,0] absorbs q_sample scales sqrt(ac_t), sqrt(1-ac_t),
    the timestep embedding channel and b1 (via the ones channel).
  - For each BN layer i: pass A computes pre-activations a~ = W' h tile-by-tile
    (PSUM) and per-tile mean/M2 via DVE bn_stats; bn_aggr + 1KB AllReduce give
    global mean/var.  Since gamma>0:  h = relu(s*(a-m)+beta) = s*relu(a + u)
    with s = gamma/std, u = beta/s - m,  so the per-channel scale s is folded
    into the NEXT layer's weights (tiny on-device op) and pass B re-runs the
    matmul and evacuates PSUM->SBUF with a single fused Relu(a+u) activation op.
    (BN absorbs the linear bias, so b2..b5 drop out exactly.)
  - Layer 6 computes pred - noise directly via a 2-matmul PSUM accumulation
    ([W6'|(-I;b6)] against [h5] and [noise;ones]) and only its bn_stats are
    shipped out; the host combines the 8 cores' per-chunk stats into the MSE.
"""
import numpy as np
import ml_dtypes

import concourse.bass as bass
import concourse.bacc as bacc
import concourse.mybir as mybir
from concourse import tile
from concourse.bass_utils import run_bass_kernel_spmd

BF16 = ml_dtypes.bfloat16
F32 = mybir.dt.float32
BF = mybir.dt.bfloat16
AOp = mybir.AluOpType
AF = mybir.ActivationFunctionType

NCORES = 8
NB = 8            # batches per core
TS = 512          # points per tile (= one PSUM bank of fp32)
BN_EPS = 1e-5
T_STEPS = 1000

# wm / wf column layout (lhsT = [C_in, C_out] slices)
COLS = {2: (0, 128, 64), 3: (128, 256, 128), 4: (256, 384, 128),
        5: (384, 448, 128), 6: (448, 451, 64)}  # start, stop, n_rows
C_OUT = {1: 64, 2: 128, 3: 128, 4: 128, 5: 64, 6: 3}


def _schedule():
    s = 0.008
    x = np.linspace(0.0, T_STEPS, T_STEPS + 1)
    ac = np.cos((x / T_STEPS + s) / (1.0 + s) * np.pi * 0.5) ** 2
    ac = ac / ac[0]
    betas = np.clip(1.0 - ac[1:] / ac[:-1], 1e-4, 2e-2)
    acp = np.cumprod(1.0 - betas)
    return np.sqrt(acp).astype(np.float32), np.sqrt(1.0 - acp).astype(np.float32)


def build_program(nc: bass.Bass, ptb: int, collective: bool = True):
    """Build the SPMD per-core program.  ptb = points per batch (N)."""
    ppc = NB * ptb                 # points per core
    nt_b = ptb // TS               # tiles per batch
    ntiles = ppc // TS
    RG = [list(range(NCORES))]

    xn_d = nc.dram_tensor("xn", [128, 2 * ptb], BF, kind="ExternalInput")
    w1eff_d = nc.dram_tensor("w1eff", [128, 128], BF, kind="ExternalInput")
    wm_d = nc.dram_tensor("wm", [128, 452], F32, kind="ExternalInput")
    w6b_d = nc.dram_tensor("w6b", [128, 4], BF, kind="ExternalInput")
    bnp_d = nc.dram_tensor("bnp", [128, 10], F32, kind="ExternalInput")
    mstats_d = nc.dram_tensor("mstats", [3, 6 * ntiles], F32, kind="ExternalOutput")

    with tile.TileContext(nc) as tc:
        with (
            tc.tile_pool(name="const", bufs=1) as cpool,
            tc.tile_pool(name="work", bufs=2) as wpool,
            tc.tile_pool(name="psum", bufs=8, space="PSUM") as pspool,
            tc.tile_pool(name="dram", bufs=2, space="DRAM") as dpool,
        ):
            H = cpool.tile([128, ppc], BF)
            xn = cpool.tile([128, 2 * ptb], BF)
            w1eff = cpool.tile([128, 128], BF)
            wm = cpool.tile([128, 452], F32)
            wf = cpool.tile([128, 452], BF)
            w6b = cpool.tile([128, 4], BF)
            bnp = cpool.tile([128, 10], F32)
            mstats = cpool.tile([3, 6 * ntiles], F32)
            epsv = cpool.tile([128, 1], F32)
            nc.vector.memset(epsv[:, :], BN_EPS)
            junk = cpool.tile([128, 1], F32)
            # pre-warm ACT table sets (Relu, Sqrt) so the table loads don't
            # attach to wait-heavy instructions (walrus per-inst wait budget)
            nc.scalar.activation(junk[:, :], epsv[:, :], AF.Relu, bias=epsv[:, :], scale=1.0)
            nc.scalar.activation(junk[:, :], epsv[:, :], AF.Sqrt, bias=epsv[:, :], scale=1.0)

            nc.sync.dma_start(xn[:, :], xn_d[:, :])
            nc.sync.dma_start(w1eff[:, :], w1eff_d[:, :])
            nc.sync.dma_start(wm[:, :], wm_d[:, :])
            nc.sync.dma_start(w6b[:, :], w6b_d[:, :])
            nc.sync.dma_start(bnp[:, :], bnp_d[:, :])

            def l1_ops(ti):
                k, tt = divmod(ti, nt_b)
                base, half = 32 * (k % 4), k // 4
                lhsT = w1eff[base:base + 8, half * 64:half * 64 + 64]
                rhs = xn[base:base + 8, half * ptb + tt * TS: half * ptb + (tt + 1) * TS]
                return lhsT, rhs, (base, 0)

            def mk_ops(i):  # layers 2..6 main matmul operands
                lo, hi, rows = COLS[i]
                def ops(ti):
                    return wf[0:rows, lo:hi], H[0:rows, ti * TS:(ti + 1) * TS], (0, 0)
                return ops

            def pass_a(opfn, C, stats):
                for ti in range(ntiles):
                    lhsT, rhs, tp = opfn(ti)
                    psa = pspool.tile([128, TS], F32, tag="ps", name=f"psa")
                    nc.tensor.matmul(psa[0:C, :], lhsT, rhs, tile_position=tp)
                    nc.vector.bn_stats(stats[0:C, 6 * ti:6 * ti + 6], psa[0:C, :])

            def pass_b(opfn, C, uv):
                for ti in range(ntiles):
                    lhsT, rhs, tp = opfn(ti)
                    psb = pspool.tile([128, TS], F32, tag="ps", name=f"psb")
                    nc.tensor.matmul(psb[0:C, :], lhsT, rhs, tile_position=tp)
                    nc.scalar.activation(
                        H[0:C, ti * TS:(ti + 1) * TS], psb[0:C, :],
                        AF.Relu, bias=uv[0:C, :], scale=1.0)

            def stats_to_su(i, C, stats):
                """bn_aggr -> AllReduce(1KB) -> per-channel scale s and bias u."""
                agg = wpool.tile([128, 2], F32, tag="agg", name="agg")
                nc.vector.bn_aggr(agg[0:C, :], stats[0:C, 0:6 * ntiles])
                m2 = wpool.tile([128, 1], F32, tag="m2", name="m2")
                nc.vector.tensor_tensor(m2[0:C, :], agg[0:C, 0:1], agg[0:C, 0:1], op=AOp.mult)
                pk = wpool.tile([128, 2], F32, tag="pk", name="pk")
                # payload: [mean/8, (var+mean^2)/8]; AR-add over 8 equal-count cores
                nc.vector.tensor_scalar(pk[0:C, 0:1], agg[0:C, 0:1], 1.0 / NCORES, None, op0=AOp.mult)
                nc.vector.tensor_scalar(pk[0:C, 1:2], agg[0:C, 1:2], m2[0:C, :], 1.0 / NCORES,
                                        op0=AOp.add, op1=AOp.mult)
                ari = dpool.tile([C, 2], F32, tag="ari", name="ari")
                aro = dpool.tile([C, 2], F32, tag="aro", name="aro")
                nc.gpsimd.dma_start(ari[:, :], pk[0:C, :])
                if collective:
                    nc.gpsimd.collective_compute(
                        "AllReduce", AOp.add, replica_groups=RG,
                        ins=[ari[:, :].opt()], outs=[aro[:, :].opt()])
                else:  # single-core timing variant: AR == copy
                    nc.gpsimd.dma_start(aro[:, :], ari[:, :])
                gb = wpool.tile([128, 2], F32, tag="gb", name="gb")
                nc.gpsimd.dma_start(gb[0:C, :], aro[:, :])
                m2g = wpool.tile([128, 1], F32, tag="m2g", name="m2g")
                nc.vector.tensor_tensor(m2g[0:C, :], gb[0:C, 0:1], gb[0:C, 0:1], op=AOp.mult)
                varv = wpool.tile([128, 1], F32, tag="varv", name="varv")
                nc.vector.tensor_tensor(varv[0:C, :], gb[0:C, 1:2], m2g[0:C, :], op=AOp.subtract)
                stdv = wpool.tile([128, 1], F32, tag="stdv", name="stdv")
                nc.scalar.activation(stdv[0:C, :], varv[0:C, :], AF.Sqrt, bias=epsv[0:C, :], scale=1.0)
                rcpv = wpool.tile([128, 1], F32, tag="rcpv", name="rcpv")
                nc.vector.reciprocal(rcpv[0:C, :], stdv[0:C, :])
                sv = wpool.tile([128, 1], F32, tag="sv", name="sv")
                nc.vector.tensor_tensor(sv[0:C, :], rcpv[0:C, :], bnp[0:C, 2 * i - 1:2 * i], op=AOp.mult)
                uv = wpool.tile([128, 1], F32, tag="uv", name="uv")
                # u = (beta/gamma)*std - mean
                nc.vector.tensor_scalar(uv[0:C, :], stdv[0:C, :], bnp[0:C, 2 * i - 2:2 * i - 1],
                                        gb[0:C, 0:1], op0=AOp.mult, op1=AOp.subtract)
                return sv, uv

            def fold(next_i, sv):
                lo, hi, rows = COLS[next_i]
                nc.vector.tensor_scalar_mul(wf[0:rows, lo:hi], wm[0:rows, lo:hi], sv[0:rows, :])

            # ---- layers 1..5 ----
            opfns = {1: l1_ops, 2: mk_ops(2), 3: mk_ops(3), 4: mk_ops(4), 5: mk_ops(5)}
            for i in range(1, 6):
                C = C_OUT[i]
                stats = wpool.tile([128, 6 * ntiles], F32, tag="stats", name=f"stats{i}")
                pass_a(opfns[i], C, stats)
                sv, uv = stats_to_su(i, C, stats)
                fold(i + 1, sv)
                # dummy ACT read of uv: pulls the DVE->ACT sync off the per-tile
                # evacuation activations (walrus per-instruction wait budget)
                nc.scalar.activation(junk[0:C, :], uv[0:C, :], AF.Relu, bias=epsv[0:C, :], scale=1.0)
                pass_b(opfns[i], C, uv)

            # ---- layer 6: pred - noise, stats only ----
            for ti in range(ntiles):
                k, tt = divmod(ti, nt_b)
                base, half = 32 * (k % 4), k // 4
                ps6 = pspool.tile([128, TS], F32, tag="ps", name="ps6")
                nc.tensor.matmul(ps6[0:3, :], wf[0:64, 448:451],
                                 H[0:64, ti * TS:(ti + 1) * TS], start=True, stop=False)
                nc.tensor.matmul(ps6[0:3, :], w6b[base:base + 4, 0:3],
                                 xn[base:base + 4, half * ptb + tt * TS: half * ptb + (tt + 1) * TS],
                                 start=False, stop=True, tile_position=(base, 0))
                nc.vector.bn_stats(mstats[0:3, 6 * ti:6 * ti + 6], ps6[0:3, :])

            nc.sync.dma_start(mstats_d[:, :], mstats[:, :])

    return nc


# ---------------- host side ----------------

def host_pack(x, t, noise, params, ptb):
    """Build the 8 per-core input maps."""
    B, N, _ = x.shape
    assert N == ptb and B == NCORES * NB
    p = {k: np.asarray(v, np.float32) for k, v in params.items()}
    sqa, sq1 = _schedule()
    s0 = sqa[np.asarray(t)]
    s1 = sq1[np.asarray(t)]
    tf = np.asarray(t).astype(np.float32)

    w1 = p['w1']  # (64, 4)

    # wm: master lhsT weights, fp32 (identical on all cores)
    wm = np.zeros((128, 452), np.float32)
    wm[0:64, 0:128] = p['w2'].T
    wm[0:128, 128:256] = p['w3'].T
    wm[0:128, 256:384] = p['w4'].T
    wm[0:128, 384:448] = p['w5'].T
    wm[0:64, 448:451] = p['w6'].T

    # w6b: [-I3; b6] at each 32-aligned base
    w6b = np.zeros((128, 4), np.float32)
    for j in range(4):
        w6b[32 * j:32 * j + 3, 0:3] = -np.eye(3, dtype=np.float32)
        w6b[32 * j + 3, 0:3] = p['b6']
    w6b = w6b.astype(BF16)

    # bnp: per layer cols [beta/gamma, gamma]
    bnp = np.zeros((128, 10), np.float32)
    for i, (g, be) in enumerate([(p['g1'], p['be1']), (p['g2'], p['be2']),
                                 (p['g3'], p['be3']), (p['g4'], p['be4']),
                                 (p['g5'], p['be5'])]):
        C = len(g)
        bnp[0:C, 2 * i] = be / g
        bnp[0:C, 2 * i + 1] = g

    wm_l = wm
    in_maps = []
    for c in range(NCORES):
        xn = np.zeros((128, 2 * ptb), np.float32)
        w1eff = np.zeros((128, 128), np.float32)
        for k in range(NB):
            b = c * NB + k
            base, half = 32 * (k % 4), k // 4
            cols = slice(half * ptb, (half + 1) * ptb)
            xn[base + 0:base + 3, cols] = noise[b].T
            xn[base + 3, cols] = 1.0
            xn[base + 4:base + 7, cols] = x[b].T
            # lhsT [8, 64]
            blk = np.zeros((8, 64), np.float32)
            blk[0:3] = s1[b] * w1[:, 0:3].T
            blk[3] = tf[b] * w1[:, 3] + p['b1']
            blk[4:7] = s0[b] * w1[:, 0:3].T
            w1eff[base:base + 8, half * 64:(half + 1) * 64] = blk
        in_maps.append({
            "xn": xn.astype(BF16),
            "w1eff": w1eff.astype(BF16),
            "wm": wm_l,
            "w6b": w6b,
            "bnp": bnp,
        })
    return in_maps


def combine_mstats(results):
    """results: list of per-core dicts with 'mstats' [3, 6*ntiles] -> scalar MSE."""
    vals = []
    for r in results:
        ms = np.asarray(r["mstats"], np.float64)         # [3, 6*nt]
        tri = ms.reshape(3, -1, 3)                        # [3, 2*nt, 3] triples
        counts, means, ctv = tri[..., 0], tri[..., 1], tri[..., 2]
        assert np.all(counts == counts.flat[0])
        vals.append(ctv / counts + means ** 2)            # E[d^2] per equal chunk
    return np.float32(np.mean(np.concatenate([v.ravel() for v in vals])))


_PROGRAMS = {}


def get_program(ptb, collective=True):
    key = (ptb, collective)
    if key not in _PROGRAMS:
        nc = bacc.Bacc(None, target_bir_lowering=False, debug=False,
                       num_devices=NCORES if collective else 1)
        build_program(nc, ptb, collective=collective)
        nc.compile()
        _PROGRAMS[key] = nc
    return _PROGRAMS[key]


def kernel(x, t, noise, params, _trace=False):
    x = np.asarray(x, np.float32)
    noise = np.asarray(noise, np.float32)
    t = np.asarray(t)
    ptb = x.shape[1]
    nc = get_program(ptb)
    in_maps = host_pack(x, t, noise, params, ptb)
    res = run_bass_kernel_spmd(nc, in_maps, core_ids=list(range(NCORES)), trace=_trace)
    mse = combine_mstats(res.results)
    if _trace:
        return mse, res
    return mse
